# revision 16
# baseline (speedup 1.0000x reference)
"""BinMLP (binarized 6-layer MLP, 784-256x5-10 + BN + softmax) on 8 TRN2 NeuronCores.

Pure data parallel: batch 65536 split 8192/core. Activations are kept
feature-major [features, batch] on chip so every layer's output feeds the next
matmul's moving operand directly.

Numerics:
  - Layer 0 consumes raw fp32 x against +-1 weights. The PE's fp32 mode is 4x
    slower and fp32r is TF32-grade, so x is split hi/lo into two fp16 tensors
    (x == hi + lo to ~2^-24) stacked along the contraction dim (K=1568). +-1
    weights are exact in fp16, so this runs at full bf16 PE rate with
    fp32-grade accuracy (HW-measured rms 2.5e-6 vs fp64, better than np fp32).
  - Hidden layers see +-1/0 activations and +-1 weights: exact in bf16/fp8,
    psum accumulation is exact integer arithmetic in fp32.
  - BN + bias folds to Sign(z + (b - mean)) per feature: since z is an exact
    integer and |b - mean| < 1, the sign decision is bit-identical to the
    reference's (z + b) - mean rounding order. Hardtanh is dropped entirely
    (clip never changes the sign the next layer consumes).
  - BN sign-safety: sign(a) of the BN scale is folded into the next layer's
    weights; hidden layers then only need the +-1 weight matmul plus bias.

Scheduling: per-engine instruction streams are in-order, so the L1->L5
cascade (matmul -> Sign on ACT -> matmul) would stall the PE every layer.
Emission is software-pipelined: tile j's 26 layer-0 matmuls are emitted in 5
groups interleaved with tile j-1's cascade stages, so the PE always has
independent work while ACT produces the next layer's signs.
"""

import os
import contextlib
import numpy as np
import ml_dtypes

import concourse.bass as bass
import concourse.tile as tile
from concourse import bacc, mybir
from concourse import bass_utils

F32 = mybir.dt.float32
F16 = mybir.dt.float16
BF16 = mybir.dt.bfloat16
FP8 = mybir.dt.float8e4
NP_FP8 = mybir.dt.np(FP8)
AF = mybir.ActivationFunctionType

N_CORES = 8
B = 65536
BC = B // N_CORES          # 8192 rows per core
BT = 512                   # batch tile (psum bank limit for fp32 output)
NJ = BC // BT              # 16 batch tiles per core
IN_F = 784
K2 = 2 * IN_F              # 1568 stacked hi/lo rows
NC0 = 12                   # full 128-row chunks of K2 (plus one 32-row tail)
NCH = 13                   # total K chunks for layer 0
HID = 256
NCLS = 10

last_results = None        # BassKernelResults of the most recent run (for test.py)


def _build_program(repeat=1, dynamic_repeat=None, mode="full", hidden="bf16",
                   pipelined=True, tailpack=True):
    nc = bacc.Bacc("TRN2", target_bir_lowering=False, debug=False)
    HW8 = FP8 if hidden == "dr8" else BF16

    xt_d = nc.dram_tensor("xt", [K2, BC], F16, kind="ExternalInput")
    w0_d = nc.dram_tensor("w0", [K2, HID], F16, kind="ExternalInput")
    wh_d = nc.dram_tensor("wh", [4, HID, HID], HW8, kind="ExternalInput")
    w5_d = nc.dram_tensor("w5", [HID, NCLS], HW8, kind="ExternalInput")
    bh_d = nc.dram_tensor("bh", [128, 10], F32, kind="ExternalInput")
    p5r_d = nc.dram_tensor("p5r", [2, 128, 40], F32, kind="ExternalInput")
    out_d = nc.dram_tensor("out", [BC, NCLS], F32, kind="ExternalOutput")

    with tile.TileContext(nc) as tc:
        with (
            tc.tile_pool(name="wpool", bufs=1) as wp,
            tc.tile_pool(name="xpool", bufs=4) as xp,
            tc.tile_pool(name="apool", bufs=3) as ap,
            tc.tile_pool(name="spool", bufs=2) as sp,
            tc.tile_pool(name="pspool", bufs=1, space="PSUM") as pp,
        ):
            # ---- resident weights / constants ----
            w0t_a = wp.tile([128, NC0, HID], F16)          # K2 rows 0..1535
            nc.sync.dma_start(
                w0t_a[:], w0_d[0:NC0 * 128, :].rearrange("(c p) m -> p c m", p=128))
            if tailpack:
                # tail rows replicated at partition offsets 0 and 32 so the two
                # K=32 tail matmuls (m0, m1) run concurrently in distinct
                # PE row-groups via tile_position
                w0t_b = wp.tile([64, HID], F16)
                nc.sync.dma_start(w0t_b[0:32, :], w0_d[NC0 * 128:K2, :])
                nc.sync.dma_start(w0t_b[32:64, :], w0_d[NC0 * 128:K2, :])
            else:
                w0t_b = wp.tile([K2 - NC0 * 128, HID], F16)
                nc.sync.dma_start(w0t_b[:], w0_d[NC0 * 128:K2, :])
            wht = wp.tile([128, 4, 2, HID], HW8)
            nc.sync.dma_start(
                wht[:], wh_d[:, :, :].rearrange("l (k p) m -> p l k m", p=128))
            w5t = wp.tile([128, 2, NCLS], HW8)
            nc.sync.dma_start(w5t[:], w5_d[:, :].rearrange("(k p) m -> p k m", p=128))
            bht = wp.tile([128, 10], F32)
            nc.sync.dma_start(bht[:], bh_d[:, :])
            a5rep = wp.tile([128, 40], F32)
            nc.sync.dma_start(a5rep[:], p5r_d[0, :, :])
            b5rep = wp.tile([128, 40], F32)
            nc.sync.dma_start(b5rep[:], p5r_d[1, :, :])

            x_first = {}    # mode=="compute": reuse tile 0's x everywhere

            def l0_gen(j, js, out):
                """Yield 5x; emits layer-0 (DMA, 26 matmuls, 2 Signs).
                Stores the act tile into out["act"]."""
                if mode == "compute" and x_first:
                    xa, xb = x_first["a"], x_first["b"]
                else:
                    xa = xp.tile([128, NC0, BT], F16, tag="xa", name=f"xa{j}")
                    nc.sync.dma_start(
                        xa[:],
                        xt_d[0:NC0 * 128, js].rearrange("(c p) n -> p c n", p=128))
                    if tailpack:
                        xb = xp.tile([64, BT], F16, tag="xb", name=f"xb{j}")
                        nc.sync.dma_start(xb[0:32, :], xt_d[NC0 * 128:K2, js])
                        nc.sync.dma_start(xb[32:64, :], xt_d[NC0 * 128:K2, js])
                    else:
                        xb = xp.tile([K2 - NC0 * 128, BT], F16, tag="xb",
                                     name=f"xb{j}")
                        nc.sync.dma_start(xb[:], xt_d[NC0 * 128:K2, js])
                    if mode == "compute" and not x_first:
                        x_first["a"], x_first["b"] = xa, xb
                if mode == "dma":
                    for _ in range(5):
                        yield
                    return
                act = ap.tile([128, 2, BT], HW8, tag="act", name=f"a0_{j}")
                out["act"] = act
                ps = {}

                def emit(m, c):
                    ms = slice(m * 128, (m + 1) * 128)
                    if c == 0:
                        ps[m] = pp.tile([128, BT], F32, tag=f"psL0_{m}",
                                        bufs=2, name=f"ps0_{m}_{j}")
                    if c < NC0:
                        nc.tensor.matmul(ps[m][:], w0t_a[:, c, ms], xa[:, c, :],
                                         start=(c == 0), stop=False)
                    elif not tailpack:
                        nc.tensor.matmul(ps[m][:], w0t_b[:, ms], xb[:],
                                         start=False, stop=True)
                    else:
                        po = m * 32
                        nc.tensor.matmul(ps[m][:],
                                         w0t_b[po:po + 32, ms], xb[po:po + 32, :],
                                         start=False, stop=True,
                                         tile_position=(po, 0))
                    if c == NCH - 1:
                        nc.scalar.activation(act[:, m, :], ps[m][:], AF.Sign,
                                             bias=bht[:, m:m + 1], scale=1.0)

                if tailpack:
                    # both K=32 tails adjacent at the end -> concurrent row-groups
                    seq = ([(m, c) for m in range(2) for c in range(NC0)]
                           + [(0, NC0), (1, NC0)])
                    groups = [seq[0:5], seq[5:10], seq[10:15], seq[15:20],
                              seq[20:26]]
                else:
                    seq = [(m, c) for m in range(2) for c in range(NCH)]
                    groups = [seq[0:6], seq[6:12], seq[12:18], seq[18:24],
                              seq[24:26]]
                for grp in groups:
                    for (m, c) in grp:
                        emit(m, c)
                    yield

            def cascade_gen(j, js, act):
                """Yield 5x; emits layers 1..4, then layer 5 + softmax + out."""
                for l in range(4):
                    nact = ap.tile([128, 2, BT], HW8, tag="act", name=f"a{l + 1}_{j}")
                    for m in range(2):
                        ms = slice(m * 128, (m + 1) * 128)
                        ps = pp.tile([128, BT], F32, tag=f"psC_{m}", bufs=1,
                                     name=f"ps{l + 1}_{m}_{j}")
                        if hidden == "dr8":
                            nc.tensor.matmul(ps[:], wht[:, l, :, ms], act[:],
                                             start=True, stop=True,
                                             perf_mode=mybir.MatmulPerfMode.DoubleRow)
                        else:
                            for k in range(2):
                                nc.tensor.matmul(ps[:], wht[:, l, k, ms],
                                                 act[:, k, :],
                                                 start=(k == 0), stop=(k == 1))
                        nc.scalar.activation(
                            nact[:, m, :], ps[:], AF.Sign,
                            bias=bht[:, (l + 1) * 2 + m:(l + 1) * 2 + m + 1],
                            scale=1.0)
                    act = nact
                    yield

                # ---- layer 5, batch-major: logits [128, 4, 10] in one bank ----
                ps5 = pp.tile([128, 40], F32, tag="ps5", bufs=2, name=f"ps5_{j}")
                n_mm = 0
                for g in range(4):
                    for k in range(2):
                        nc.tensor.matmul(ps5[:, g * 10:(g + 1) * 10],
                                         act[:, k, g * 128:(g + 1) * 128],
                                         w5t[:, k, :],
                                         start=(n_mm == 0), stop=(n_mm == 7))
                        n_mm += 1
                pst = sp.tile([128, 40], F32, tag="lgt", name=f"lg_{j}")
                nc.vector.tensor_mul(pst[:], ps5[:], a5rep[:])
                nc.vector.tensor_add(pst[:], pst[:], b5rep[:])
                if mode == "nosm":
                    nc.sync.dma_start(out_d[js, :], pst[:])  # timing-only
                    yield
                    return

                ot = sp.tile([128, 40], F32, tag="ot", name=f"ot_{j}")
                nmx = sp.tile([128, 4], F32, tag="nmx", name=f"nmx_{j}")
                ssum = sp.tile([128, 4], F32, tag="ssum", name=f"ssum_{j}")
                rcp = sp.tile([128, 4], F32, tag="rcp", name=f"rcp_{j}")
                for g in range(4):
                    gs = slice(g * 10, (g + 1) * 10)
                    nc.vector.reduce_max(nmx[:, g:g + 1], pst[:, gs],
                                         axis=mybir.AxisListType.X, negate=True)
                    nc.scalar.activation(ot[:, gs], pst[:, gs], AF.Exp,
                                         bias=nmx[:, g:g + 1], scale=1.0,
                                         accum_out=ssum[:, g:g + 1])
                    nc.vector.reciprocal(rcp[:, g:g + 1], ssum[:, g:g + 1])
                    nc.vector.tensor_scalar_mul(ot[:, gs], ot[:, gs],
                                                rcp[:, g:g + 1])
                nc.sync.dma_start(
                    out_d[js, :].rearrange("(g p) f -> p g f", p=128), ot[:])
                yield

            loop_ctx = (tc.For_i(0, dynamic_repeat, 1) if dynamic_repeat
                        else contextlib.nullcontext())
            with loop_ctx:
                for rep in range(repeat):
                    prev = None
                    for j in range(NJ):
                        js = slice(j * BT, (j + 1) * BT)
                        uj = rep * NJ + j
                        out = {}
                        g0 = l0_gen(uj, js, out)
                        for _ in range(5):
                            next(g0)
                            if prev is not None and pipelined:
                                next(prev, None)
                        if mode == "dma":
                            prev = None
                            continue
                        if prev is not None and not pipelined:
                            for _ in prev:
                                pass
                        prev = cascade_gen(uj, js, out["act"])
                    if prev is not None:
                        for _ in prev:
                            pass

    nc.compile()
    return nc


def _prepare_in_maps(x, Ws, bs, gammas, betas, means, variances, hidden="bf16"):
    f32 = np.float32

    x = np.asarray(x, dtype=f32).reshape(B, IN_F)
    Ws = [np.asarray(w, dtype=f32) for w in Ws]
    bs = [np.asarray(v, dtype=f32) for v in bs]
    gammas = [np.asarray(v, dtype=f32) for v in gammas]
    betas = [np.asarray(v, dtype=f32) for v in betas]
    means = [np.asarray(v, dtype=f32) for v in means]
    variances = [np.asarray(v, dtype=f32) for v in variances]

    # ---- fold BN ----
    # a = gamma * rsqrt(var+eps) (>0 in practice); sign(a) folds into the next
    # layer's weights, so each hidden layer reduces to Sign(z + bias_eff) with
    # bias_eff = (b - mean) + beta/a  (exactly b - mean when beta == 0, which
    # preserves the reference's fp32 rounding order bit-for-bit).
    a_l = [gammas[i] / np.sqrt(variances[i] + f32(1e-5)) for i in range(6)]
    s_l = [np.where(a == 0, f32(1.0), np.sign(a)).astype(f32) for a in a_l]

    Wb = [np.sign(w).astype(f32) for w in Ws]
    Weff = [Wb[0]] + [Wb[i] * s_l[i - 1][None, :] for i in range(1, 6)]

    bias_h = []
    for i in range(5):
        c = bs[i] - means[i]  # fp32 RN, matches reference when beta==0
        if np.any(betas[i] != 0):
            c = (c.astype(np.float64) + betas[i].astype(np.float64)
                 / a_l[i].astype(np.float64)).astype(f32)
        bias_h.append(c)  # sign(a*u+beta) = sign(a) * sign(u + beta/a)
    c5 = bs[5] - means[5]
    a5 = a_l[5]
    bias5 = (a5.astype(np.float64) * c5.astype(np.float64)
             + betas[5].astype(np.float64)).astype(f32)

    # packed per-partition hidden biases: col = layer*2 + mtile
    bh = np.zeros((128, 10), dtype=f32)
    for l in range(5):
        for m in range(2):
            bh[:, l * 2 + m] = bias_h[l][m * 128:(m + 1) * 128]
    # [2, 128, 40]: class-affine replicated batch-major (col = g*10 + class)
    p5r = np.zeros((2, 128, 40), dtype=f32)
    p5r[0] = np.tile(a5, 4)[None, :]
    p5r[1] = np.tile(bias5, 4)[None, :]

    # ---- weights in device layouts ----
    np8 = NP_FP8 if hidden == "dr8" else ml_dtypes.bfloat16
    w0T = np.ascontiguousarray(Weff[0].T)                    # [784, 256]
    w0 = np.concatenate([w0T, w0T], axis=0).astype(np.float16)   # [1568, 256]
    wh = np.stack([np.ascontiguousarray(Weff[l].T) for l in range(1, 5)], axis=0)
    wh = wh.astype(np8)                                      # [4, 256, 256]
    w5 = np.ascontiguousarray(Weff[5].T).astype(np8)         # [256, 10]

    # ---- per-core x shards: fp16 hi/lo split, feature-major ----
    in_maps = []
    for c in range(N_CORES):
        xs = x[c * BC:(c + 1) * BC]                          # [8192, 784] f32
        hi = xs.astype(np.float16)
        lo = (xs - hi.astype(f32)).astype(np.float16)
        xT2 = np.ascontiguousarray(
            np.concatenate([hi.T, lo.T], axis=0))            # [1568, 8192] f16
        in_maps.append({"xt": xT2, "w0": w0, "wh": wh, "w5": w5,
                        "bh": bh, "p5r": p5r})
    return in_maps


def kernel(x, Ws, bs, gammas, betas, means, variances):
    global last_results
    in_maps = _prepare_in_maps(x, Ws, bs, gammas, betas, means, variances)
    nc = _build_program()
    last_results = bass_utils.run_bass_kernel_spmd(
        nc, in_maps, core_ids=list(range(N_CORES)),
        tmpdir=os.environ.get("BASS_KERNEL_TMPDIR"))
    return np.concatenate([r["out"] for r in last_results.results], axis=0)


# revision 17
# speedup vs baseline: 1.5058x; 1.5058x over previous
"""BinMLP (binarized 6-layer MLP, 784-256x5-10 + BN + softmax) on 8 TRN2 NeuronCores.

Pure data parallel: batch 65536 split 8192/core. Activations are kept
feature-major [features, batch] on chip so every layer's output feeds the next
matmul's moving operand directly.

Numerics:
  - Layer 0 consumes raw fp32 x against +-1 weights. The PE's fp32 mode is 4x
    slower and fp32r is TF32-grade, so x is split hi/lo into two fp16 tensors
    (x == hi + lo to ~2^-24) stacked along the contraction dim (K=1568). +-1
    weights are exact in fp16, so this runs at full bf16 PE rate with
    fp32-grade accuracy (HW-measured rms 2.5e-6 vs fp64, better than np fp32).
  - Hidden layers see +-1/0 activations and +-1 weights: exact in bf16/fp8,
    psum accumulation is exact integer arithmetic in fp32.
  - BN + bias folds to Sign(z + (b - mean)) per feature: since z is an exact
    integer and |b - mean| < 1, the sign decision is bit-identical to the
    reference's (z + b) - mean rounding order. Hardtanh is dropped entirely
    (clip never changes the sign the next layer consumes).
  - BN sign-safety: sign(a) of the BN scale is folded into the next layer's
    weights; hidden layers then only need the +-1 weight matmul plus bias.

Scheduling: per-engine instruction streams are in-order, so the L1->L5
cascade (matmul -> Sign on ACT -> matmul) would stall the PE every layer.
Emission is software-pipelined: tile j's 26 layer-0 matmuls are emitted in 5
groups interleaved with tile j-1's cascade stages, so the PE always has
independent work while ACT produces the next layer's signs.
"""

import os
import contextlib
import numpy as np
import ml_dtypes

import concourse.bass as bass
import concourse.tile as tile
from concourse import bacc, mybir
from concourse import bass_utils

F32 = mybir.dt.float32
F16 = mybir.dt.float16
BF16 = mybir.dt.bfloat16
FP8 = mybir.dt.float8e4
NP_FP8 = mybir.dt.np(FP8)
AF = mybir.ActivationFunctionType

N_CORES = 8
B = 65536
BC = B // N_CORES          # 8192 rows per core
BT = 512                   # batch tile (psum bank limit for fp32 output)
NJ = BC // BT              # 16 batch tiles per core
IN_F = 784
K2 = 2 * IN_F              # 1568 stacked hi/lo rows
NC0 = 12                   # full 128-row chunks of K2 (plus one 32-row tail)
NCH = 13                   # total K chunks for layer 0
HID = 256
NCLS = 10

last_results = None        # BassKernelResults of the most recent run (for test.py)


def _build_program(repeat=1, dynamic_repeat=None, mode="full", hidden="bf16",
                   pipelined=True, tailpack=False, groups5=False):
    nc = bacc.Bacc("TRN2", target_bir_lowering=False, debug=False)
    HW8 = FP8 if hidden == "dr8" else BF16

    xt_d = nc.dram_tensor("xt", [K2, BC], F16, kind="ExternalInput")
    w0_d = nc.dram_tensor("w0", [K2, HID], F16, kind="ExternalInput")
    wh_d = nc.dram_tensor("wh", [4, HID, HID], HW8, kind="ExternalInput")
    w5_d = nc.dram_tensor("w5", [HID, NCLS], HW8, kind="ExternalInput")
    bh_d = nc.dram_tensor("bh", [128, 10], F32, kind="ExternalInput")
    p5r_d = nc.dram_tensor("p5r", [2, 128, 40], F32, kind="ExternalInput")
    out_d = nc.dram_tensor("out", [BC, NCLS], F32, kind="ExternalOutput")

    with tile.TileContext(nc) as tc:
        with (
            tc.tile_pool(name="wpool", bufs=1) as wp,
            tc.tile_pool(name="xpool", bufs=4) as xp,
            tc.tile_pool(name="apool", bufs=3) as ap,
            tc.tile_pool(name="spool", bufs=2) as sp,
            tc.tile_pool(name="pspool", bufs=1, space="PSUM") as pp,
        ):
            # ---- resident weights / constants ----
            w0t_a = wp.tile([128, NC0, HID], F16)          # K2 rows 0..1535
            nc.sync.dma_start(
                w0t_a[:], w0_d[0:NC0 * 128, :].rearrange("(c p) m -> p c m", p=128))
            if tailpack:
                # tail rows replicated at partition offsets 0 and 32 so the two
                # K=32 tail matmuls (m0, m1) run concurrently in distinct
                # PE row-groups via tile_position
                w0t_b = wp.tile([64, HID], F16)
                nc.sync.dma_start(w0t_b[0:32, :], w0_d[NC0 * 128:K2, :])
                nc.sync.dma_start(w0t_b[32:64, :], w0_d[NC0 * 128:K2, :])
            else:
                w0t_b = wp.tile([K2 - NC0 * 128, HID], F16)
                nc.sync.dma_start(w0t_b[:], w0_d[NC0 * 128:K2, :])
            wht = wp.tile([128, 4, 2, HID], HW8)
            nc.sync.dma_start(
                wht[:], wh_d[:, :, :].rearrange("l (k p) m -> p l k m", p=128))
            w5t = wp.tile([128, 2, NCLS], HW8)
            nc.sync.dma_start(w5t[:], w5_d[:, :].rearrange("(k p) m -> p k m", p=128))
            bht = wp.tile([128, 10], F32)
            nc.sync.dma_start(bht[:], bh_d[:, :])
            a5rep = wp.tile([128, 40], F32)
            nc.sync.dma_start(a5rep[:], p5r_d[0, :, :])
            b5rep = wp.tile([128, 40], F32)
            nc.sync.dma_start(b5rep[:], p5r_d[1, :, :])

            x_first = {}    # mode=="compute": reuse tile 0's x everywhere

            def l0_gen(j, js, out):
                """Yield 5x; emits layer-0 (DMA, 26 matmuls, 2 Signs).
                Stores the act tile into out["act"]."""
                if mode == "compute" and x_first:
                    xa, xb = x_first["a"], x_first["b"]
                else:
                    xa = xp.tile([128, NC0, BT], F16, tag="xa", name=f"xa{j}")
                    nc.sync.dma_start(
                        xa[:],
                        xt_d[0:NC0 * 128, js].rearrange("(c p) n -> p c n", p=128))
                    if tailpack:
                        xb = xp.tile([64, BT], F16, tag="xb", name=f"xb{j}")
                        nc.sync.dma_start(xb[0:32, :], xt_d[NC0 * 128:K2, js])
                        nc.sync.dma_start(xb[32:64, :], xt_d[NC0 * 128:K2, js])
                    else:
                        xb = xp.tile([K2 - NC0 * 128, BT], F16, tag="xb",
                                     name=f"xb{j}")
                        nc.sync.dma_start(xb[:], xt_d[NC0 * 128:K2, js])
                    if mode == "compute" and not x_first:
                        x_first["a"], x_first["b"] = xa, xb
                if mode == "dma":
                    for _ in range(5):
                        yield
                    return
                act = ap.tile([128, 2, BT], HW8, tag="act", name=f"a0_{j}")
                out["act"] = act
                ps = {}

                def emit(m, c):
                    ms = slice(m * 128, (m + 1) * 128)
                    if c == 0:
                        ps[m] = pp.tile([128, BT], F32, tag=f"psL0_{m}",
                                        bufs=2, name=f"ps0_{m}_{j}")
                    if c < NC0:
                        nc.tensor.matmul(ps[m][:], w0t_a[:, c, ms], xa[:, c, :],
                                         start=(c == 0), stop=False)
                    elif not tailpack:
                        nc.tensor.matmul(ps[m][:], w0t_b[:, ms], xb[:],
                                         start=False, stop=True)
                    else:
                        po = m * 32
                        nc.tensor.matmul(ps[m][:],
                                         w0t_b[po:po + 32, ms], xb[po:po + 32, :],
                                         start=False, stop=True,
                                         tile_position=(po, 0))
                    if c == NCH - 1:
                        nc.scalar.activation(act[:, m, :], ps[m][:], AF.Sign,
                                             bias=bht[:, m:m + 1], scale=1.0)

                if tailpack:
                    # both K=32 tails adjacent at the end -> concurrent row-groups
                    seq = ([(m, c) for m in range(2) for c in range(NC0)]
                           + [(0, NC0), (1, NC0)])
                    groups = [seq[0:5], seq[5:10], seq[10:15], seq[15:20],
                              seq[20:26]]
                elif groups5:
                    seq = [(m, c) for m in range(2) for c in range(NCH)]
                    groups = [seq[0:5], seq[5:10], seq[10:15], seq[15:20],
                              seq[20:26]]
                else:
                    seq = [(m, c) for m in range(2) for c in range(NCH)]
                    groups = [seq[0:6], seq[6:12], seq[12:18], seq[18:24],
                              seq[24:26]]
                for grp in groups:
                    for (m, c) in grp:
                        emit(m, c)
                    yield

            def cascade_gen(j, js, act):
                """Yield 5x; emits layers 1..4, then layer 5 + softmax + out."""
                for l in range(4):
                    nact = ap.tile([128, 2, BT], HW8, tag="act", name=f"a{l + 1}_{j}")
                    for m in range(2):
                        ms = slice(m * 128, (m + 1) * 128)
                        ps = pp.tile([128, BT], F32, tag=f"psC_{m}", bufs=1,
                                     name=f"ps{l + 1}_{m}_{j}")
                        if hidden == "dr8":
                            nc.tensor.matmul(ps[:], wht[:, l, :, ms], act[:],
                                             start=True, stop=True,
                                             perf_mode=mybir.MatmulPerfMode.DoubleRow)
                        else:
                            for k in range(2):
                                nc.tensor.matmul(ps[:], wht[:, l, k, ms],
                                                 act[:, k, :],
                                                 start=(k == 0), stop=(k == 1))
                        nc.scalar.activation(
                            nact[:, m, :], ps[:], AF.Sign,
                            bias=bht[:, (l + 1) * 2 + m:(l + 1) * 2 + m + 1],
                            scale=1.0)
                    act = nact
                    yield

                # ---- layer 5, batch-major: logits [128, 4, 10] in one bank ----
                ps5 = pp.tile([128, 40], F32, tag="ps5", bufs=2, name=f"ps5_{j}")
                n_mm = 0
                for g in range(4):
                    for k in range(2):
                        nc.tensor.matmul(ps5[:, g * 10:(g + 1) * 10],
                                         act[:, k, g * 128:(g + 1) * 128],
                                         w5t[:, k, :],
                                         start=(n_mm == 0), stop=(n_mm == 7))
                        n_mm += 1
                pst = sp.tile([128, 40], F32, tag="lgt", name=f"lg_{j}")
                nc.vector.tensor_mul(pst[:], ps5[:], a5rep[:])
                nc.vector.tensor_add(pst[:], pst[:], b5rep[:])
                if mode == "nosm":
                    nc.sync.dma_start(out_d[js, :], pst[:])  # timing-only
                    yield
                    return

                ot = sp.tile([128, 40], F32, tag="ot", name=f"ot_{j}")
                nmx = sp.tile([128, 4], F32, tag="nmx", name=f"nmx_{j}")
                ssum = sp.tile([128, 4], F32, tag="ssum", name=f"ssum_{j}")
                rcp = sp.tile([128, 4], F32, tag="rcp", name=f"rcp_{j}")
                for g in range(4):
                    gs = slice(g * 10, (g + 1) * 10)
                    nc.vector.reduce_max(nmx[:, g:g + 1], pst[:, gs],
                                         axis=mybir.AxisListType.X, negate=True)
                    nc.scalar.activation(ot[:, gs], pst[:, gs], AF.Exp,
                                         bias=nmx[:, g:g + 1], scale=1.0,
                                         accum_out=ssum[:, g:g + 1])
                    nc.vector.reciprocal(rcp[:, g:g + 1], ssum[:, g:g + 1])
                    nc.vector.tensor_scalar_mul(ot[:, gs], ot[:, gs],
                                                rcp[:, g:g + 1])
                nc.sync.dma_start(
                    out_d[js, :].rearrange("(g p) f -> p g f", p=128), ot[:])
                yield

            loop_ctx = (tc.For_i(0, dynamic_repeat, 1) if dynamic_repeat
                        else contextlib.nullcontext())
            with loop_ctx:
                for rep in range(repeat):
                    prev = None
                    for j in range(NJ):
                        js = slice(j * BT, (j + 1) * BT)
                        uj = rep * NJ + j
                        out = {}
                        g0 = l0_gen(uj, js, out)
                        for _ in range(5):
                            next(g0)
                            if prev is not None and pipelined:
                                next(prev, None)
                        if mode == "dma":
                            prev = None
                            continue
                        if prev is not None and not pipelined:
                            for _ in prev:
                                pass
                        prev = cascade_gen(uj, js, out["act"])
                    if prev is not None:
                        for _ in prev:
                            pass

    nc.compile()
    return nc


def _prepare_in_maps(x, Ws, bs, gammas, betas, means, variances, hidden="bf16"):
    f32 = np.float32

    x = np.asarray(x, dtype=f32).reshape(B, IN_F)
    Ws = [np.asarray(w, dtype=f32) for w in Ws]
    bs = [np.asarray(v, dtype=f32) for v in bs]
    gammas = [np.asarray(v, dtype=f32) for v in gammas]
    betas = [np.asarray(v, dtype=f32) for v in betas]
    means = [np.asarray(v, dtype=f32) for v in means]
    variances = [np.asarray(v, dtype=f32) for v in variances]

    # ---- fold BN ----
    # a = gamma * rsqrt(var+eps) (>0 in practice); sign(a) folds into the next
    # layer's weights, so each hidden layer reduces to Sign(z + bias_eff) with
    # bias_eff = (b - mean) + beta/a  (exactly b - mean when beta == 0, which
    # preserves the reference's fp32 rounding order bit-for-bit).
    a_l = [gammas[i] / np.sqrt(variances[i] + f32(1e-5)) for i in range(6)]
    s_l = [np.where(a == 0, f32(1.0), np.sign(a)).astype(f32) for a in a_l]

    Wb = [np.sign(w).astype(f32) for w in Ws]
    Weff = [Wb[0]] + [Wb[i] * s_l[i - 1][None, :] for i in range(1, 6)]

    bias_h = []
    for i in range(5):
        c = bs[i] - means[i]  # fp32 RN, matches reference when beta==0
        if np.any(betas[i] != 0):
            c = (c.astype(np.float64) + betas[i].astype(np.float64)
                 / a_l[i].astype(np.float64)).astype(f32)
        bias_h.append(c)  # sign(a*u+beta) = sign(a) * sign(u + beta/a)
    c5 = bs[5] - means[5]
    a5 = a_l[5]
    bias5 = (a5.astype(np.float64) * c5.astype(np.float64)
             + betas[5].astype(np.float64)).astype(f32)

    # packed per-partition hidden biases: col = layer*2 + mtile
    bh = np.zeros((128, 10), dtype=f32)
    for l in range(5):
        for m in range(2):
            bh[:, l * 2 + m] = bias_h[l][m * 128:(m + 1) * 128]
    # [2, 128, 40]: class-affine replicated batch-major (col = g*10 + class)
    p5r = np.zeros((2, 128, 40), dtype=f32)
    p5r[0] = np.tile(a5, 4)[None, :]
    p5r[1] = np.tile(bias5, 4)[None, :]

    # ---- weights in device layouts ----
    np8 = NP_FP8 if hidden == "dr8" else ml_dtypes.bfloat16
    w0T = np.ascontiguousarray(Weff[0].T)                    # [784, 256]
    w0 = np.concatenate([w0T, w0T], axis=0).astype(np.float16)   # [1568, 256]
    wh = np.stack([np.ascontiguousarray(Weff[l].T) for l in range(1, 5)], axis=0)
    wh = wh.astype(np8)                                      # [4, 256, 256]
    w5 = np.ascontiguousarray(Weff[5].T).astype(np8)         # [256, 10]

    # ---- per-core x shards: fp16 hi/lo split, feature-major ----
    in_maps = []
    for c in range(N_CORES):
        xs = x[c * BC:(c + 1) * BC]                          # [8192, 784] f32
        hi = xs.astype(np.float16)
        lo = (xs - hi.astype(f32)).astype(np.float16)
        xT2 = np.ascontiguousarray(
            np.concatenate([hi.T, lo.T], axis=0))            # [1568, 8192] f16
        in_maps.append({"xt": xT2, "w0": w0, "wh": wh, "w5": w5,
                        "bh": bh, "p5r": p5r})
    return in_maps


def kernel(x, Ws, bs, gammas, betas, means, variances):
    global last_results
    in_maps = _prepare_in_maps(x, Ws, bs, gammas, betas, means, variances)
    nc = _build_program()
    last_results = bass_utils.run_bass_kernel_spmd(
        nc, in_maps, core_ids=list(range(N_CORES)),
        tmpdir=os.environ.get("BASS_KERNEL_TMPDIR"))
    return np.concatenate([r["out"] for r in last_results.results], axis=0)
